# revision 11
# baseline (speedup 1.0000x reference)
"""Distributed ISTFT kernel for Trainium2 (8 NeuronCores, Bass/Tile).

Math (matches the jax reference):
  z: [2, 513, T] one-sided spectrum (real/imag), T = 8192 frames.
  Hermitian extension + ifft(1024) + window + overlap-add (hop 256) +
  divide by overlapped window sum + trim 512 each side -> [2, 2096896].

Structure:
  * real(ifft) frame: frame[n] = c[n] + s[n], frame[1024-n] = c[n] - s[n]
    where c = cos-transform of zr (bins 0..512), s = -sin-transform of
    zi (bins 1..511).  Computing c and s separately (512 samples each)
    HALVES the matmul columns vs the dense 1024-wide iDFT.
  * Window w (periodic Hann, symmetric: w[1024-n] = w[n]) and the
    interior 1/overlap-sum (= 0.5 exactly for hop N/4) fold into the
    C/S matrices host-side.  Column m of Cw/Sw <-> sample m+1.
  * zr[512] (Nyquist, rank-1 cos row) rides a K=1 tap matmul into C.
  * Overlap-add: out[b, r] = Wf_{b+3}[r] + Wf_{b+2}[256+r]
    + Wf_{b+1}[512+r] + Wf_b[768+r] (slot-indexed) becomes 4 psum-read
    combines (U = C+S, V = C-S at mirrored/descending columns) on DVE,
    then a 3-add tree on GPSIMD -> bf16 out tile.
  * imag(ifft)[n, t] = (zi[0,t] + (-1)^n zi[512,t]) / N (rank-2) is
    channel 1: tiny K=4 matmuls, evicted via the Scalar (ACT) engine.
  * The first/last 256 output samples see a 3-frame window sum; they are
    rescaled on the host (512 samples/channel, elementwise).
  * Everything streams in bf16 (tolerance 2e-2, achieved ~3e-3).
  * Output blocks tile as 8x125 + 24 with a 3-slot frame halo so every
    overlap-add read stays inside one 128-partition psum tile.
  * Frame axis sharded: 1024 output blocks/core + 3-frame input halo,
    no cross-core communication.
"""

import numpy as np
import ml_dtypes

N_FFT = 1024
HOP = 256
T_FRAMES = 8192
N_CORES = 8
F_SLOTS = 1027  # frame slots per core: 1024 owned blocks need slots t..t+3
NB = 1024       # output blocks computed per core (core 7 uses 1023)
TILE_B = 125    # ch0 blocks per psum tile (last tile: 24)

BF16 = ml_dtypes.bfloat16

_CACHE = {}


def _mats():
    """Cw/Sw [512, 512] and tapr [512]: col m <-> sample m+1, f64."""
    n_s = np.arange(1, 513, dtype=np.float64)[None, :]
    k = np.arange(512, dtype=np.float64)[:, None]
    g = np.where(k == 0, 1.0, 2.0)
    Cw = g * np.cos(2.0 * np.pi * k * n_s / N_FFT)
    Sw = -2.0 * np.sin(2.0 * np.pi * (k + 1.0) * n_s / N_FFT)
    Sw[511, :] = 0.0  # row 511 <-> x row 1023 = zr512 (handled by tapr)
    tapr = np.cos(np.pi * n_s[0])  # (-1)^(m+1): zr512 cos row, g=1
    return Cw, Sw, tapr


def _build_nc():
    from contextlib import ExitStack

    import concourse.tile as tile
    from concourse import bacc, mybir

    f32 = mybir.dt.float32
    bf = mybir.dt.bfloat16

    nc = bacc.Bacc("TRN2", target_bir_lowering=False, debug=False,
                   num_devices=N_CORES)

    x_d = nc.dram_tensor("x", [1026, F_SLOTS], bf, kind="ExternalInput")
    c_d = nc.dram_tensor("cw", [512, 512], bf, kind="ExternalInput")
    s_d = nc.dram_tensor("sw", [512, 512], bf, kind="ExternalInput")
    r_d = nc.dram_tensor("tapr", [1, 512], bf, kind="ExternalInput")
    t_d = nc.dram_tensor("taps", [8, 256], bf, kind="ExternalInput")
    o_d = nc.dram_tensor("out", [2, NB, 256], bf, kind="ExternalOutput")

    with tile.TileContext(nc) as tc, ExitStack() as ctx:
        big = ctx.enter_context(tc.tile_pool(name="big", bufs=1))
        sml = ctx.enter_context(tc.tile_pool(name="sml", bufs=1))
        stg = ctx.enter_context(tc.tile_pool(name="stg", bufs=2))
        ustg = ctx.enter_context(tc.tile_pool(name="ustg", bufs=2))
        psC = ctx.enter_context(tc.tile_pool(name="psC", bufs=3, space="PSUM"))
        psS = ctx.enter_context(tc.tile_pool(name="psS", bufs=3, space="PSUM"))
        ps1p = ctx.enter_context(tc.tile_pool(name="ps1p", bufs=2, space="PSUM"))

        # small inputs on the gpsimd (SWDGE) queue
        tpu = sml.tile([4, 256], bf, tag="tpu")
        nc.gpsimd.dma_start(out=tpu[:], in_=t_d.ap()[0:4, :])
        tpv = sml.tile([4, 256], bf, tag="tpv")
        nc.gpsimd.dma_start(out=tpv[:], in_=t_d.ap()[4:8, :])
        taprt = sml.tile([1, 512], bf, tag="taprt")
        nc.gpsimd.dma_start(out=taprt[:], in_=r_d.ap())
        tw = sml.tile([1, F_SLOTS], bf, tag="tw")
        nc.gpsimd.dma_start(out=tw[:], in_=x_d.ap()[1023:1024, :])
        tu = sml.tile([4, NB], bf, tag="tu")
        tv = sml.tile([4, NB], bf, tag="tv")
        for q in range(4):
            nc.gpsimd.dma_start(out=tu[q:q + 1, :],
                                in_=x_d.ap()[1024:1025, 3 - q:3 - q + NB])
            nc.gpsimd.dma_start(out=tv[q:q + 1, :],
                                in_=x_d.ap()[1025:1026, 3 - q:3 - q + NB])

        # big streams: x chunks on sync (C-order interleaved with S-order),
        # Cw/Sw chunks on scalar
        xs = [None] * 8
        for k in (0, 4, 1, 5, 2, 6, 3, 7):
            xk = big.tile([128, F_SLOTS], bf, tag=f"xs{k}")
            nc.sync.dma_start(out=xk[:],
                              in_=x_d.ap()[128 * k:128 * (k + 1), :])
            xs[k] = xk
        cw = [None] * 4
        sw = [None] * 4
        for k in range(4):
            ck = big.tile([128, 512], bf, tag=f"cw{k}")
            nc.scalar.dma_start(out=ck[:], in_=c_d.ap()[128 * k:128 * (k + 1), :])
            cw[k] = ck
            sk = big.tile([128, 512], bf, tag=f"sw{k}")
            nc.scalar.dma_start(out=sk[:], in_=s_d.ap()[128 * k:128 * (k + 1), :])
            sw[k] = sk

        # HAM warm-up: dummy matmuls on a memset tile while the first data
        # chunks are in flight, so the PE clock-gate is 8/8 for the stream.
        wtile = sml.tile([4, 256], bf, tag="wtile")
        nc.vector.memset(wtile[:], 0.0)
        wps = ps1p.tile([128, 256], f32, tag="ps1", name="warm")
        for i in range(12):
            nc.tensor.matmul(wps[:], lhsT=wtile[:, 0:128], rhs=wtile[:],
                             start=(i == 0), stop=(i == 11))

        def ch1_group(tt):
            ps1 = ps1p.tile([128, 256], f32, tag="ps1", name=f"ps1_{tt}")
            nc.tensor.matmul(ps1[:], lhsT=tu[:, tt * 128:tt * 128 + 128],
                             rhs=tpu[:], start=True, stop=False)
            nc.tensor.matmul(ps1[:], lhsT=tv[:, tt * 128:tt * 128 + 128],
                             rhs=tpv[:], start=False, stop=True)
            o = stg.tile([128, 256], bf, tag="o1", name=f"o1_{tt}")
            nc.scalar.copy(o[:], ps1[:])
            nc.scalar.dma_start(
                out=o_d.ap()[1:2, tt * 128:(tt + 1) * 128, :], in_=o[:])

        def ch0_tile(j):
            B = TILE_B if j < 8 else NB - 8 * TILE_B
            M = B + 3
            base = TILE_B * j
            Cp = psC.tile([128, 512], f32, tag="psC", name=f"psC_{j}")
            Sp = psS.tile([128, 512], f32, tag="psS", name=f"psS_{j}")
            for k in range(4):
                nc.tensor.matmul(Cp[0:M, :], lhsT=xs[k][:, base:base + M],
                                 rhs=cw[k][:], start=(k == 0), stop=False)
                nc.tensor.matmul(Sp[0:M, :], lhsT=xs[4 + k][:, base:base + M],
                                 rhs=sw[k][:], start=(k == 0), stop=(k == 3))
            nc.tensor.matmul(Cp[0:M, :], lhsT=tw[:, base:base + M],
                             rhs=taprt[:], start=False, stop=True)
            # U[t, m] = Wf_t[m+1], V[t, m] = Wf_t[1023-m].  A TensorTensor
            # op may read only ONE psum input, so ACT drains C first.
            Cs = ustg.tile([128, 512], bf, tag="Cs", name=f"Cs_{j}")
            nc.scalar.copy(Cs[0:M, :], Cp[0:M, :])
            U = ustg.tile([128, 512], bf, tag="U", name=f"U_{j}")
            nc.vector.tensor_add(U[0:M, :], Cs[0:M, :], Sp[0:M, :])
            V = ustg.tile([128, 512], bf, tag="V", name=f"V_{j}")
            nc.vector.tensor_sub(V[0:M, :], Cs[0:M, :], Sp[0:M, :])
            # partition shifts via SBUF->SBUF DMA (engines need 32-aligned
            # partition bases; DMA does not)
            c1 = stg.tile([TILE_B, 256], bf, tag="c1", name=f"c1_{j}")
            nc.sync.dma_start(out=c1[0:B, :], in_=U[3:3 + B, 0:256])
            c2 = stg.tile([TILE_B, 256], bf, tag="c2", name=f"c2_{j}")
            nc.sync.dma_start(out=c2[0:B, :], in_=U[2:2 + B, 255:511])
            c3 = stg.tile([TILE_B, 256], bf, tag="c3", name=f"c3_{j}")
            nc.scalar.dma_start(out=c3[0:B, :], in_=V[1:1 + B, 256:512])
            # out[p, r] = c1[p, r-1] + c2[p, r] + c3[p, 255-r] + V[p, 255-r]
            a1 = stg.tile([TILE_B, 256], bf, tag="a1", name=f"a1_{j}")
            nc.gpsimd.tensor_add(a1[0:B, :], c2[0:B, :], c3[0:B, 255::-1])
            a2 = stg.tile([TILE_B, 256], bf, tag="a2", name=f"a2_{j}")
            nc.gpsimd.tensor_add(a2[0:B, :], a1[0:B, :], V[0:B, 255::-1])
            o = stg.tile([TILE_B, 256], bf, tag="o0", name=f"o0_{j}")
            nc.gpsimd.tensor_add(o[0:B, 1:256], a2[0:B, 1:256], c1[0:B, 0:255])
            nc.gpsimd.tensor_copy(o[0:B, 0:1], a2[0:B, 0:1])
            queue = nc.sync if (j % 2 == 0) else nc.scalar
            queue.dma_start(out=o_d.ap()[0:1, base:base + B, :], in_=o[0:B, :])

        for j in range(9):
            ch0_tile(j)
            if j >= 1:
                ch1_group(j - 1)

    nc.compile()
    return nc


def _inputs_for_cores(z: np.ndarray, window: np.ndarray):
    mats = _CACHE.get("mats")
    if mats is None:
        mats = _mats()
        _CACHE["mats"] = mats
    Cw, Sw, tapr = mats
    w64 = window.astype(np.float64)
    wn = w64[1:513] * (0.5 / N_FFT)  # window * interior 1/ws, col-aligned
    cwb = np.ascontiguousarray((Cw * wn[None, :]).astype(BF16))
    swb = np.ascontiguousarray((Sw * wn[None, :]).astype(BF16))
    taprb = np.ascontiguousarray((tapr * wn)[None, :].astype(BF16))
    # ch1 taps: rows 0-3 = w-quarters * 0.5/N, rows 4-7 = same * (-1)^r
    w4 = w64.reshape(4, 256) * (0.5 / N_FFT)
    alt = 1.0 - 2.0 * (np.arange(256) % 2)
    taps = np.ascontiguousarray(
        np.concatenate([w4, w4 * alt[None, :]], 0).astype(BF16))

    in_maps = []
    for c in range(N_CORES):
        G = 1024 * c - 1  # global frame index of slot 0
        X = np.zeros((1026, F_SLOTS), np.float32)
        lo, hi = max(0, G), min(T_FRAMES, G + F_SLOTS)
        s0, s1 = lo - G, hi - G
        X[0:512, s0:s1] = z[0, 0:512, lo:hi]
        X[512:1023, s0:s1] = z[1, 1:512, lo:hi]
        X[1023, s0:s1] = z[0, 512, lo:hi]
        X[1024, s0:s1] = z[1, 0, lo:hi]
        X[1025, s0:s1] = z[1, 512, lo:hi]
        in_maps.append({
            "x": X.astype(BF16),
            "cw": cwb,
            "sw": swb,
            "tapr": taprb,
            "taps": taps,
        })
    return in_maps


def kernel(z: np.ndarray, window: np.ndarray) -> np.ndarray:
    from concourse.bass_utils import run_bass_kernel_spmd

    z = np.asarray(z, dtype=np.float32)
    window = np.asarray(window, dtype=np.float32)

    nc = _CACHE.get("nc")
    if nc is None:
        nc = _build_nc()
        _CACHE["nc"] = nc

    in_maps = _inputs_for_cores(z, window)
    res = run_bass_kernel_spmd(nc, in_maps, list(range(N_CORES)))

    parts = []
    for c in range(N_CORES):
        nb = NB if c < N_CORES - 1 else NB - 1
        o = res.results[c]["out"]  # [2, NB, 256] bf16
        parts.append(o[:, :nb, :].reshape(2, -1).astype(np.float32))
    out = np.concatenate(parts, axis=1)
    # edge fixup: first/last 256 samples see a 3-frame window sum
    # (2 - w[768+r] and 2 - w[r]); the kernel normalized by 2 everywhere.
    w64 = window.astype(np.float64)
    out[:, :256] *= (2.0 / (2.0 - w64[768:1024])).astype(np.float32)
    out[:, -256:] *= (2.0 / (2.0 - w64[0:256])).astype(np.float32)
    return np.ascontiguousarray(out)


# revision 12
# speedup vs baseline: 1.1251x; 1.1251x over previous
"""Distributed ISTFT kernel for Trainium2 (8 NeuronCores, Bass/Tile).

Math (matches the jax reference):
  z: [2, 513, T] one-sided spectrum (real/imag), T = 8192 frames.
  Hermitian extension + ifft(1024) + window + overlap-add (hop 256) +
  divide by overlapped window sum + trim 512 each side -> [2, 2096896].

Structure:
  * real(ifft) frame: frame[n] = c[n] + s[n], frame[1024-n] = c[n] - s[n]
    where c = cos-transform of zr (bins 0..512), s = -sin-transform of
    zi (bins 1..511).  Computing c and s separately (512 samples each)
    HALVES the matmul columns vs the dense 1024-wide iDFT.
  * Window w (periodic Hann, symmetric) and the interior 1/overlap-sum
    (= 0.5 exactly for hop N/4) fold into Cw/Sw host-side.  Column m of
    Cw/Sw <-> sample m+1.  zr[512] (Nyquist) rides a K=1 tap matmul.
  * Per 125-block tile: psum C [B+3, 512], S [B+3, 512]; ACT drains C to
    SBUF (a TensorTensor reads at most one psum input), DVE forms
    U = C+S (Wf samples 1..512) and V = C-S (Wf samples 1023..512,
    mirrored); SBUF->SBUF DMAs produce the +1/+2/+3 partition-shifted
    copies (compute engines require 32-aligned partition bases, DMA does
    not); GPSIMD does the 3-add overlap-add tree -> bf16 out tile.
  * imag(ifft)[n,t] = (zi[0,t] + (-1)^n zi[512,t])/N (rank-2) is ch1:
    8 tiny K=4 matmul groups run FIRST (doubling as PE HAM warm-up),
    then their 2 psum banks are released to the ch0 pipeline.
  * First/last 256 output samples (3-frame window sum) rescaled on host.
  * Everything streams bf16 (tolerance 2e-2, achieved ~4e-3).
  * Frame axis sharded: 1024 output blocks/core + 3-frame input halo,
    no cross-core communication.
"""

import numpy as np
import ml_dtypes

N_FFT = 1024
HOP = 256
T_FRAMES = 8192
N_CORES = 8
F_SLOTS = 1027  # frame slots per core: 1024 owned blocks need slots t..t+3
NB = 1024       # output blocks computed per core (core 7 uses 1023)
TILE_B = 125    # ch0 blocks per psum tile (last tile: 24)

BF16 = ml_dtypes.bfloat16

_CACHE = {}


def _mats():
    """Cw/Sw [512, 512] and tapr [512]: col m <-> sample m+1, f64."""
    n_s = np.arange(1, 513, dtype=np.float64)[None, :]
    k = np.arange(512, dtype=np.float64)[:, None]
    g = np.where(k == 0, 1.0, 2.0)
    Cw = g * np.cos(2.0 * np.pi * k * n_s / N_FFT)
    Sw = -2.0 * np.sin(2.0 * np.pi * (k + 1.0) * n_s / N_FFT)
    Sw[511, :] = 0.0  # row 511 <-> x row 1023 = zr512 (handled by tapr)
    tapr = np.cos(np.pi * n_s[0])  # (-1)^(m+1): zr512 cos row, g=1
    return Cw, Sw, tapr


def _build_nc():
    from contextlib import ExitStack

    import concourse.tile as tile
    from concourse import bacc, mybir

    f32 = mybir.dt.float32
    bf = mybir.dt.bfloat16

    nc = bacc.Bacc("TRN2", target_bir_lowering=False, debug=False,
                   num_devices=N_CORES)

    # x rows: 0..511 zr0..511, 512..1022 zi1..511, 1023 zr512,
    # 1024 zi0, 1025 zi512, 1026..1029 tu (zi0 shifted), 1030..1033 tv
    x_d = nc.dram_tensor("x", [1034, F_SLOTS], bf, kind="ExternalInput")
    c_d = nc.dram_tensor("cw", [512, 512], bf, kind="ExternalInput")
    s_d = nc.dram_tensor("sw", [512, 512], bf, kind="ExternalInput")
    r_d = nc.dram_tensor("tapr", [1, 512], bf, kind="ExternalInput")
    t_d = nc.dram_tensor("taps", [8, 256], bf, kind="ExternalInput")
    o_d = nc.dram_tensor("out", [2, NB, 256], bf, kind="ExternalOutput")

    with tile.TileContext(nc) as tc, ExitStack() as ctx:
        big = ctx.enter_context(tc.tile_pool(name="big", bufs=1))
        sml = ctx.enter_context(tc.tile_pool(name="sml", bufs=1))
        stg = ctx.enter_context(tc.tile_pool(name="stg", bufs=4))
        ustg = ctx.enter_context(tc.tile_pool(name="ustg", bufs=3))

        # small inputs on the gpsimd (SWDGE) queue
        tpu = sml.tile([4, 256], bf, tag="tpu")
        nc.gpsimd.dma_start(out=tpu[:], in_=t_d.ap()[0:4, :])
        tpv = sml.tile([4, 256], bf, tag="tpv")
        nc.gpsimd.dma_start(out=tpv[:], in_=t_d.ap()[4:8, :])
        taprt = sml.tile([1, 512], bf, tag="taprt")
        nc.gpsimd.dma_start(out=taprt[:], in_=r_d.ap())
        tw = sml.tile([1, F_SLOTS], bf, tag="tw")
        nc.gpsimd.dma_start(out=tw[:], in_=x_d.ap()[1023:1024, :])

        # big streams: x chunks on sync, Cw/Sw chunks on scalar (HWDGE)
        xs = [None] * 8
        tut = sml.tile([4, NB], bf, tag="tut")
        tvt = sml.tile([4, NB], bf, tag="tvt")
        first = True
        for k in (0, 4, 1, 5, 2, 6, 3, 7):
            xk = big.tile([128, F_SLOTS], bf, tag=f"xs{k}")
            nc.sync.dma_start(out=xk[:],
                              in_=x_d.ap()[128 * k:128 * (k + 1), :])
            xs[k] = xk
            if first:
                nc.sync.dma_start(out=tut[:], in_=x_d.ap()[1026:1030, 0:NB])
                first = False
        cw = [None] * 4
        sw = [None] * 4
        for k in range(4):
            ck = big.tile([128, 512], bf, tag=f"cw{k}")
            nc.scalar.dma_start(out=ck[:], in_=c_d.ap()[128 * k:128 * (k + 1), :])
            cw[k] = ck
            if k == 0:
                nc.scalar.dma_start(out=tvt[:], in_=x_d.ap()[1030:1034, 0:NB])
            sk = big.tile([128, 512], bf, tag=f"sw{k}")
            nc.scalar.dma_start(out=sk[:], in_=s_d.ap()[128 * k:128 * (k + 1), :])
            sw[k] = sk

        # --- phase 1: HAM warm-up + all of channel 1 (2 psum banks,
        # released before the ch0 pipeline claims all 8)
        with tc.tile_pool(name="ps1p", bufs=2, space="PSUM") as ps1p:
            wtile = sml.tile([4, 256], bf, tag="wtile")
            nc.vector.memset(wtile[:], 0.0)
            wps = ps1p.tile([128, 256], f32, tag="ps1", name="warm")
            for i in range(12):
                nc.tensor.matmul(wps[:], lhsT=wtile[:, 0:128], rhs=wtile[:],
                                 start=(i == 0), stop=(i == 11))
            for tt in range(8):
                ps1 = ps1p.tile([128, 256], f32, tag="ps1", name=f"ps1_{tt}")
                nc.tensor.matmul(ps1[:], lhsT=tut[:, tt * 128:tt * 128 + 128],
                                 rhs=tpu[:], start=True, stop=False)
                nc.tensor.matmul(ps1[:], lhsT=tvt[:, tt * 128:tt * 128 + 128],
                                 rhs=tpv[:], start=False, stop=True)
                o1 = stg.tile([128, 256], bf, tag="o1", name=f"o1_{tt}")
                nc.vector.tensor_copy(o1[:], ps1[:])
                nc.gpsimd.dma_start(
                    out=o_d.ap()[1:2, tt * 128:(tt + 1) * 128, :], in_=o1[:])

        # --- phase 2: channel 0, 9 overlapping tiles
        psC = ctx.enter_context(tc.tile_pool(name="psC", bufs=4, space="PSUM"))
        psS = ctx.enter_context(tc.tile_pool(name="psS", bufs=4, space="PSUM"))

        def ch0_tile(j):
            B = TILE_B if j < 8 else NB - 8 * TILE_B
            M = B + 3
            base = TILE_B * j
            Cp = psC.tile([128, 512], f32, tag="psC", name=f"psC_{j}")
            Sp = psS.tile([128, 512], f32, tag="psS", name=f"psS_{j}")
            for k in range(4):
                nc.tensor.matmul(Cp[0:M, :], lhsT=xs[k][:, base:base + M],
                                 rhs=cw[k][:], start=(k == 0), stop=False)
                nc.tensor.matmul(Sp[0:M, :], lhsT=xs[4 + k][:, base:base + M],
                                 rhs=sw[k][:], start=(k == 0), stop=(k == 3))
            nc.tensor.matmul(Cp[0:M, :], lhsT=tw[:, base:base + M],
                             rhs=taprt[:], start=False, stop=True)
            # U[t, m] = Wf_t[m+1], V[t, m] = Wf_t[1023-m].  A TensorTensor
            # op may read only ONE psum input, so ACT drains C first.
            Cs = ustg.tile([128, 512], bf, tag="Cs", name=f"Cs_{j}")
            nc.scalar.copy(Cs[0:M, :], Cp[0:M, :])
            U = ustg.tile([128, 512], bf, tag="U", name=f"U_{j}")
            nc.vector.tensor_add(U[0:M, :], Cs[0:M, :], Sp[0:M, :])
            V = ustg.tile([128, 512], bf, tag="V", name=f"V_{j}")
            nc.vector.tensor_sub(V[0:M, :], Cs[0:M, :], Sp[0:M, :])
            # partition shifts via SBUF->SBUF DMA (engines need 32-aligned
            # partition bases; DMA does not).  c1 col 0 stays 0 so the
            # final add is uniform over r=0..255.
            c1 = stg.tile([TILE_B, 257], bf, tag="c1", name=f"c1_{j}")
            nc.vector.memset(c1[0:B, 0:1], 0.0)
            nc.sync.dma_start(out=c1[0:B, 1:257], in_=U[3:3 + B, 0:256])
            c2 = stg.tile([TILE_B, 256], bf, tag="c2", name=f"c2_{j}")
            nc.sync.dma_start(out=c2[0:B, :], in_=U[2:2 + B, 255:511])
            c3 = stg.tile([TILE_B, 256], bf, tag="c3", name=f"c3_{j}")
            nc.scalar.dma_start(out=c3[0:B, :], in_=V[1:1 + B, 256:512])
            # out[p, r] = c1[p, r] + c2[p, r] + c3[p, 255-r] + V[p, 255-r]
            a1 = stg.tile([TILE_B, 256], bf, tag="a1", name=f"a1_{j}")
            nc.gpsimd.tensor_add(a1[0:B, :], c2[0:B, :], c3[0:B, 255::-1])
            a2 = stg.tile([TILE_B, 256], bf, tag="a2", name=f"a2_{j}")
            nc.gpsimd.tensor_add(a2[0:B, :], a1[0:B, :], V[0:B, 255::-1])
            o = stg.tile([TILE_B, 256], bf, tag="o0", name=f"o0_{j}")
            nc.gpsimd.tensor_add(o[0:B, :], a2[0:B, :], c1[0:B, 0:256])
            nc.sync.dma_start(out=o_d.ap()[0:1, base:base + B, :], in_=o[0:B, :])

        for j in range(9):
            ch0_tile(j)

    nc.compile()
    return nc


def _inputs_for_cores(z: np.ndarray, window: np.ndarray):
    mats = _CACHE.get("mats")
    if mats is None:
        mats = _mats()
        _CACHE["mats"] = mats
    Cw, Sw, tapr = mats
    w64 = window.astype(np.float64)
    wn = w64[1:513] * (0.5 / N_FFT)  # window * interior 1/ws, col-aligned
    cwb = np.ascontiguousarray((Cw * wn[None, :]).astype(BF16))
    swb = np.ascontiguousarray((Sw * wn[None, :]).astype(BF16))
    taprb = np.ascontiguousarray((tapr * wn)[None, :].astype(BF16))
    # ch1 taps: rows 0-3 = w-quarters * 0.5/N, rows 4-7 = same * (-1)^r
    w4 = w64.reshape(4, 256) * (0.5 / N_FFT)
    alt = 1.0 - 2.0 * (np.arange(256) % 2)
    taps = np.ascontiguousarray(
        np.concatenate([w4, w4 * alt[None, :]], 0).astype(BF16))

    in_maps = []
    for c in range(N_CORES):
        G = 1024 * c - 1  # global frame index of slot 0
        X = np.zeros((1034, F_SLOTS), np.float32)
        lo, hi = max(0, G), min(T_FRAMES, G + F_SLOTS)
        s0, s1 = lo - G, hi - G
        X[0:512, s0:s1] = z[0, 0:512, lo:hi]
        X[512:1023, s0:s1] = z[1, 1:512, lo:hi]
        X[1023, s0:s1] = z[0, 512, lo:hi]
        X[1024, s0:s1] = z[1, 0, lo:hi]
        X[1025, s0:s1] = z[1, 512, lo:hi]
        for q in range(4):  # pre-shifted zi0/zi512 rows for the ch1 lhsT
            X[1026 + q, 0:NB] = X[1024, 3 - q:3 - q + NB]
            X[1030 + q, 0:NB] = X[1025, 3 - q:3 - q + NB]
        in_maps.append({
            "x": X.astype(BF16),
            "cw": cwb,
            "sw": swb,
            "tapr": taprb,
            "taps": taps,
        })
    return in_maps


def kernel(z: np.ndarray, window: np.ndarray) -> np.ndarray:
    from concourse.bass_utils import run_bass_kernel_spmd

    z = np.asarray(z, dtype=np.float32)
    window = np.asarray(window, dtype=np.float32)

    nc = _CACHE.get("nc")
    if nc is None:
        nc = _build_nc()
        _CACHE["nc"] = nc

    in_maps = _inputs_for_cores(z, window)
    res = run_bass_kernel_spmd(nc, in_maps, list(range(N_CORES)))

    parts = []
    for c in range(N_CORES):
        nb = NB if c < N_CORES - 1 else NB - 1
        o = res.results[c]["out"]  # [2, NB, 256] bf16
        parts.append(o[:, :nb, :].reshape(2, -1).astype(np.float32))
    out = np.concatenate(parts, axis=1)
    # edge fixup: first/last 256 samples see a 3-frame window sum
    # (2 - w[768+r] and 2 - w[r]); the kernel normalized by 2 everywhere.
    w64 = window.astype(np.float64)
    out[:, :256] *= (2.0 / (2.0 - w64[768:1024])).astype(np.float32)
    out[:, -256:] *= (2.0 / (2.0 - w64[0:256])).astype(np.float32)
    return np.ascontiguousarray(out)


# revision 13
# speedup vs baseline: 1.3688x; 1.2166x over previous
"""Distributed ISTFT kernel for Trainium2 (8 NeuronCores, Bass/Tile).

Math (matches the jax reference):
  z: [2, 513, T] one-sided spectrum (real/imag), T = 8192 frames.
  Hermitian extension + ifft(1024) + window + overlap-add (hop 256) +
  divide by overlapped window sum + trim 512 each side -> [2, 2096896].

Key folds used here:
  * real(ifft) = A^T @ X where A [1024(k), 1024(n)] packs the cos rows for
    zr bins 0..512 and sin rows for zi bins 1..511; X packs those z rows.
  * imag(ifft)[n, t] = (zi[0,t] + (-1)^n zi[512,t]) / N  (rank-2).
  * Output sample m = 256*b + r; block b = sum_{q=0..3} wf_{b-q}[256q+r].
    Folding window * A into the stationary operand gives
    O^T[t, r] = sum_q X[:, t+3-q]^T @ Aw_q directly -- the overlap-add and
    windowing ride inside the matmul.
  * The overlapped window sum for the periodic Hann window at hop N/4 is
    EXACTLY 2.0 everywhere except the first/last 256 output samples, so
    the 1/ws normalization folds into A as a global *0.5 (and into the
    ch1 taps); the two edge blocks are rescaled on the host (512 samples
    per channel, elementwise).
  * Everything streams in bf16 (tolerance is 2e-2; achieved ~3e-3),
    which halves HBM traffic and enables FWL on the PE.
  * Channel 1 (rank-2) runs FIRST with host-pre-shifted zi0/zi512 rows,
    doubling as the PE HAM warm-up while the big streams land.
  * Frame axis is sharded 1024 output blocks/core with a 3-frame input
    halo, so no cross-core communication is needed at all.
"""

import numpy as np
import ml_dtypes

N_FFT = 1024
HOP = 256
T_FRAMES = 8192
N_CORES = 8
F_SLOTS = 1027  # frame slots per core: 1024 owned blocks need slots t..t+3
NB = 1024       # output blocks computed per core (core 7 uses 1023)

BF16 = ml_dtypes.bfloat16

_CACHE = {}


def _amat() -> np.ndarray:
    """A [1024(kappa), 1024(n)]: ifft cos/sin weights, f64 (pre-fold)."""
    n = np.arange(N_FFT, dtype=np.float64)[None, :]
    k = np.arange(513, dtype=np.float64)[:, None]
    g = np.full((513, 1), 2.0)
    g[0, 0] = 1.0
    g[512, 0] = 1.0
    C = (g / N_FFT) * np.cos(2.0 * np.pi * k * n / N_FFT)
    k2 = np.arange(1, 512, dtype=np.float64)[:, None]
    S = (-2.0 / N_FFT) * np.sin(2.0 * np.pi * k2 * n / N_FFT)
    return np.concatenate([C, S], 0)


def _build_nc():
    from contextlib import ExitStack

    import concourse.tile as tile
    from concourse import bacc, mybir

    f32 = mybir.dt.float32
    bf = mybir.dt.bfloat16

    nc = bacc.Bacc("TRN2", target_bir_lowering=False, debug=False,
                   num_devices=N_CORES)

    # x rows: 0..512 zr, 513..1023 zi1..511, 1024 zi0, 1025 zi512,
    # 1026..1029 tu (zi0 pre-shifted), 1030..1033 tv (zi512 pre-shifted)
    x_d = nc.dram_tensor("x", [1034, F_SLOTS], bf, kind="ExternalInput")
    a_d = nc.dram_tensor("awn", [1024, 1024], bf, kind="ExternalInput")
    t_d = nc.dram_tensor("taps", [8, 256], bf, kind="ExternalInput")
    o_d = nc.dram_tensor("out", [2, NB, 256], bf, kind="ExternalOutput")

    with tile.TileContext(nc) as tc, ExitStack() as ctx:
        big = ctx.enter_context(tc.tile_pool(name="big", bufs=1))
        sml = ctx.enter_context(tc.tile_pool(name="sml", bufs=1))
        osb = ctx.enter_context(tc.tile_pool(name="osb", bufs=8))

        # small inputs on the gpsimd (SWDGE) queue
        tpu = sml.tile([4, 256], bf, tag="tpu")
        nc.gpsimd.dma_start(out=tpu[:], in_=t_d.ap()[0:4, :])
        tpv = sml.tile([4, 256], bf, tag="tpv")
        nc.gpsimd.dma_start(out=tpv[:], in_=t_d.ap()[4:8, :])

        # big streams: x chunks on sync, Aw chunks on scalar (both HWDGE);
        # the small ch1 lhsT tiles ride early on the same queues.
        xs = []
        tut = sml.tile([4, NB], bf, tag="tut")
        tvt = sml.tile([4, NB], bf, tag="tvt")
        for k in range(8):
            xk = big.tile([128, F_SLOTS], bf, tag=f"xs{k}")
            nc.sync.dma_start(out=xk[:],
                              in_=x_d.ap()[128 * k:128 * (k + 1), :])
            xs.append(xk)
            if k == 0:
                nc.sync.dma_start(out=tut[:], in_=x_d.ap()[1026:1030, 0:NB])
        aw = []
        for k in range(8):
            ak = big.tile([128, N_FFT], bf, tag=f"aw{k}")
            nc.scalar.dma_start(out=ak[:],
                                in_=a_d.ap()[128 * k:128 * (k + 1), :])
            aw.append(ak)
            if k == 0:
                nc.scalar.dma_start(out=tvt[:], in_=x_d.ap()[1030:1034, 0:NB])

        # --- phase 1: HAM warm-up + all of channel 1.  The dummy matmuls
        # keep the PE busy from ~7us; the ch1 groups continue the busy
        # window on real work while the big streams land, so the ch0
        # stream below runs entirely at the warm 2.4 GHz clock.
        with tc.tile_pool(name="ps1p", bufs=2, space="PSUM") as ps1p:
            wtile = sml.tile([4, 256], bf, tag="wtile")
            nc.vector.memset(wtile[:], 0.0)
            wps = ps1p.tile([128, 256], f32, tag="ps1", name="warm")
            for i in range(12):
                nc.tensor.matmul(wps[:], lhsT=wtile[:, 0:128], rhs=wtile[:],
                                 start=(i == 0), stop=(i == 11))
            for tt in range(8):
                ps1 = ps1p.tile([128, 256], f32, tag="ps1", name=f"ps1_{tt}")
                nc.tensor.matmul(ps1[:], lhsT=tut[:, tt * 128:tt * 128 + 128],
                                 rhs=tpu[:], start=True, stop=False)
                nc.tensor.matmul(ps1[:], lhsT=tvt[:, tt * 128:tt * 128 + 128],
                                 rhs=tpv[:], start=False, stop=True)
                o1 = osb.tile([128, 256], bf, tag="o1", name=f"o1_{tt}")
                nc.vector.tensor_copy(o1[:], ps1[:])
                nc.gpsimd.dma_start(
                    out=o_d.ap()[1:2, tt * 128:(tt + 1) * 128, :], in_=o1[:])

        # --- phase 2: channel 0, k-outer accumulation in two psum sweeps
        ps0p = ctx.enter_context(tc.tile_pool(name="ps0p", bufs=6,
                                              space="PSUM"))

        def evict(ps, tt, queue):
            o = osb.tile([128, 256], bf, tag="o0", name=f"o0_{tt}")
            nc.vector.tensor_copy(o[:], ps[:])
            queue.dma_start(
                out=o_d.ap()[0:1, tt * 128:(tt + 1) * 128, :], in_=o[:])

        def sweep(tts, evict_queues):
            pss = {
                tt: ps0p.tile([128, 256], f32, tag="ps0", name=f"ps0_{tt}")
                for tt in tts
            }
            for k in range(8):
                for tt in tts:
                    for q in range(4):
                        off = tt * 128 + 3 - q
                        nc.tensor.matmul(
                            pss[tt][:],
                            lhsT=xs[k][:, off:off + 128],
                            rhs=aw[k][:, 256 * q:256 * (q + 1)],
                            start=(k == 0 and q == 0),
                            stop=(k == 7 and q == 3))
            for i, tt in enumerate(tts):
                evict(pss[tt], tt, evict_queues[i % len(evict_queues)])

        sweep([0, 1, 2, 3, 4, 5], [nc.sync, nc.scalar])
        sweep([6, 7], [nc.sync, nc.scalar])

    nc.compile()
    return nc


def _inputs_for_cores(z: np.ndarray, window: np.ndarray):
    amat = _CACHE.get("amat")
    if amat is None:
        amat = _amat()
        _CACHE["amat"] = amat
    w64 = window.astype(np.float64)
    # window and the interior 1/ws (= 0.5) folded into A host-side
    awn = np.ascontiguousarray((amat * (w64 * 0.5)[None, :]).astype(BF16))
    # ch1 taps: rows 0-3 = w-quarters * 0.5/N, rows 4-7 = same * (-1)^r
    w4 = w64.reshape(4, 256) * (0.5 / N_FFT)
    alt = 1.0 - 2.0 * (np.arange(256) % 2)
    taps = np.ascontiguousarray(
        np.concatenate([w4, w4 * alt[None, :]], 0).astype(BF16))

    in_maps = []
    for c in range(N_CORES):
        G = 1024 * c - 1  # global frame index of slot 0
        X = np.zeros((1034, F_SLOTS), np.float32)
        lo, hi = max(0, G), min(T_FRAMES, G + F_SLOTS)
        s0, s1 = lo - G, hi - G
        X[0:513, s0:s1] = z[0, :, lo:hi]
        X[513:1024, s0:s1] = z[1, 1:512, lo:hi]
        X[1024, s0:s1] = z[1, 0, lo:hi]
        X[1025, s0:s1] = z[1, 512, lo:hi]
        for q in range(4):  # pre-shifted zi0/zi512 rows for the ch1 lhsT
            X[1026 + q, 0:NB] = X[1024, 3 - q:3 - q + NB]
            X[1030 + q, 0:NB] = X[1025, 3 - q:3 - q + NB]
        in_maps.append({
            "x": X.astype(BF16),
            "awn": awn,
            "taps": taps,
        })
    return in_maps


def kernel(z: np.ndarray, window: np.ndarray) -> np.ndarray:
    from concourse.bass_utils import run_bass_kernel_spmd

    z = np.asarray(z, dtype=np.float32)
    window = np.asarray(window, dtype=np.float32)

    nc = _CACHE.get("nc")
    if nc is None:
        nc = _build_nc()
        _CACHE["nc"] = nc

    in_maps = _inputs_for_cores(z, window)
    res = run_bass_kernel_spmd(nc, in_maps, list(range(N_CORES)))

    parts = []
    for c in range(N_CORES):
        nb = NB if c < N_CORES - 1 else NB - 1
        o = res.results[c]["out"]  # [2, NB, 256] bf16
        parts.append(o[:, :nb, :].reshape(2, -1).astype(np.float32))
    out = np.concatenate(parts, axis=1)
    # edge fixup: first/last 256 samples see a 3-frame window sum
    # (2 - w[768+r] and 2 - w[r]); the kernel normalized by 2 everywhere.
    w64 = window.astype(np.float64)
    out[:, :256] *= (2.0 / (2.0 - w64[768:1024])).astype(np.float32)
    out[:, -256:] *= (2.0 / (2.0 - w64[0:256])).astype(np.float32)
    return np.ascontiguousarray(out)


# revision 14
# speedup vs baseline: 1.4380x; 1.0506x over previous
"""Distributed ISTFT kernel for Trainium2 (8 NeuronCores, Bass/Tile).

Math (matches the jax reference):
  z: [2, 513, T] one-sided spectrum (real/imag), T = 8192 frames.
  Hermitian extension + ifft(1024) + window + overlap-add (hop 256) +
  divide by overlapped window sum + trim 512 each side -> [2, 2096896].

Key folds used here:
  * real(ifft) = A^T @ X where A [1024(k), 1024(n)] packs the cos rows for
    zr bins 0..512 and sin rows for zi bins 1..511; X packs those z rows.
  * imag(ifft)[n, t] = (zi[0,t] + (-1)^n zi[512,t]) / N  (rank-2).
  * Output sample m = 256*b + r; block b = sum_{q=0..3} wf_{b-q}[256q+r].
    Folding window * A into the stationary operand gives
    O^T[t, r] = sum_q X[:, t+3-q]^T @ Aw_q directly -- the overlap-add and
    windowing ride inside the matmul.
  * The overlapped window sum for the periodic Hann window at hop N/4 is
    EXACTLY 2.0 everywhere except the first/last 256 output samples, so
    the 1/ws normalization folds into A as a global *0.5 (and into the
    ch1 taps); the two edge blocks are rescaled on the host (512 samples
    per channel, elementwise).
  * Everything streams in bf16 (tolerance is 2e-2; achieved ~3e-3),
    which halves HBM traffic and enables FWL on the PE.
  * Channel 1 (rank-2) runs FIRST with host-pre-shifted zi0/zi512 rows,
    doubling as the PE HAM warm-up while the big streams land.
  * Frame axis is sharded 1024 output blocks/core with a 3-frame input
    halo, so no cross-core communication is needed at all.
"""

import numpy as np
import ml_dtypes

N_FFT = 1024
HOP = 256
T_FRAMES = 8192
N_CORES = 8
F_SLOTS = 1027  # frame slots per core: 1024 owned blocks need slots t..t+3
NB = 1024       # output blocks computed per core (core 7 uses 1023)

BF16 = ml_dtypes.bfloat16

_CACHE = {}


def _amat() -> np.ndarray:
    """A [1024(kappa), 1024(n)]: ifft cos/sin weights, f64 (pre-fold)."""
    n = np.arange(N_FFT, dtype=np.float64)[None, :]
    k = np.arange(513, dtype=np.float64)[:, None]
    g = np.full((513, 1), 2.0)
    g[0, 0] = 1.0
    g[512, 0] = 1.0
    C = (g / N_FFT) * np.cos(2.0 * np.pi * k * n / N_FFT)
    k2 = np.arange(1, 512, dtype=np.float64)[:, None]
    S = (-2.0 / N_FFT) * np.sin(2.0 * np.pi * k2 * n / N_FFT)
    return np.concatenate([C, S], 0)


def _build_nc():
    from contextlib import ExitStack

    import concourse.tile as tile
    from concourse import bacc, mybir

    f32 = mybir.dt.float32
    bf = mybir.dt.bfloat16

    nc = bacc.Bacc("TRN2", target_bir_lowering=False, debug=False,
                   num_devices=N_CORES)

    # x rows: 0..512 zr, 513..1023 zi1..511, 1024 zi0, 1025 zi512,
    # 1026..1029 tu (zi0 pre-shifted), 1030..1033 tv (zi512 pre-shifted)
    x_d = nc.dram_tensor("x", [1034, F_SLOTS], bf, kind="ExternalInput")
    a_d = nc.dram_tensor("awn", [1024, 1024], bf, kind="ExternalInput")
    t_d = nc.dram_tensor("taps", [8, 256], bf, kind="ExternalInput")
    o_d = nc.dram_tensor("out", [2, NB, 256], bf, kind="ExternalOutput")

    with tile.TileContext(nc) as tc, ExitStack() as ctx:
        big = ctx.enter_context(tc.tile_pool(name="big", bufs=1))
        sml = ctx.enter_context(tc.tile_pool(name="sml", bufs=1))
        osb = ctx.enter_context(tc.tile_pool(name="osb", bufs=8))

        # small inputs on the gpsimd (SWDGE) queue
        tpu = sml.tile([4, 256], bf, tag="tpu")
        nc.gpsimd.dma_start(out=tpu[:], in_=t_d.ap()[0:4, :])
        tpv = sml.tile([4, 256], bf, tag="tpv")
        nc.gpsimd.dma_start(out=tpv[:], in_=t_d.ap()[4:8, :])

        # big streams: x chunks on sync, Aw chunks on scalar (both HWDGE);
        # the small ch1 lhsT tiles ride early on the same queues.
        xs = []
        tut = sml.tile([4, NB], bf, tag="tut")
        tvt = sml.tile([4, NB], bf, tag="tvt")
        for k in range(8):
            xk = big.tile([128, F_SLOTS], bf, tag=f"xs{k}")
            nc.sync.dma_start(out=xk[:],
                              in_=x_d.ap()[128 * k:128 * (k + 1), :])
            xs.append(xk)
            if k == 0:
                nc.sync.dma_start(out=tut[:], in_=x_d.ap()[1026:1030, 0:NB])
        aw = []
        for k in range(8):
            ak = big.tile([128, N_FFT], bf, tag=f"aw{k}")
            nc.scalar.dma_start(out=ak[:],
                                in_=a_d.ap()[128 * k:128 * (k + 1), :])
            aw.append(ak)
            if k == 0:
                nc.scalar.dma_start(out=tvt[:], in_=x_d.ap()[1030:1034, 0:NB])

        # HAM warm-up: dummy matmuls on a memset tile sized to end right
        # when the first data chunks land (~10.5us), so the PE clock-gate
        # flips to 8/8 just as the real stream begins and never re-cools.
        ps0p = ctx.enter_context(tc.tile_pool(name="ps0p", bufs=6,
                                              space="PSUM"))
        ps1p = ctx.enter_context(tc.tile_pool(name="ps1p", bufs=2,
                                              space="PSUM"))
        wtile = sml.tile([4, 256], bf, tag="wtile")
        nc.vector.memset(wtile[:], 0.0)
        wps = ps1p.tile([128, 256], f32, tag="ps1", name="warm")
        for i in range(17):
            nc.tensor.matmul(wps[:], lhsT=wtile[:, 0:128], rhs=wtile[:],
                             start=(i == 0), stop=(i == 16))

        def evict(ps, tt, ch, queue):
            o = osb.tile([128, 256], bf, tag=f"o{ch}", name=f"o{ch}_{tt}")
            nc.vector.tensor_copy(o[:], ps[:])
            queue.dma_start(
                out=o_d.ap()[ch:ch + 1, tt * 128:(tt + 1) * 128, :], in_=o[:])

        def ch1_group(tt):
            ps1 = ps1p.tile([128, 256], f32, tag="ps1", name=f"ps1_{tt}")
            nc.tensor.matmul(ps1[:], lhsT=tut[:, tt * 128:tt * 128 + 128],
                             rhs=tpu[:], start=True, stop=False)
            nc.tensor.matmul(ps1[:], lhsT=tvt[:, tt * 128:tt * 128 + 128],
                             rhs=tpv[:], start=False, stop=True)
            evict(ps1, tt, 1, nc.gpsimd)

        # channel 0: k-outer accumulation in two psum sweeps; the tiny
        # channel-1 groups fill the PE's DMA-pacing gaps
        def sweep(tts, ch1_sched, evict_queues):
            pss = {
                tt: ps0p.tile([128, 256], f32, tag="ps0", name=f"ps0_{tt}")
                for tt in tts
            }
            for k in range(8):
                for tt in tts:
                    for q in range(4):
                        off = tt * 128 + 3 - q
                        nc.tensor.matmul(
                            pss[tt][:],
                            lhsT=xs[k][:, off:off + 128],
                            rhs=aw[k][:, 256 * q:256 * (q + 1)],
                            start=(k == 0 and q == 0),
                            stop=(k == 7 and q == 3))
                for c1 in ch1_sched.get(k, []):
                    ch1_group(c1)
            for i, tt in enumerate(tts):
                evict(pss[tt], tt, 0, evict_queues[i % len(evict_queues)])

        sweep([0, 1, 2, 3, 4, 5], {k: [k - 2] for k in range(2, 8)},
              [nc.sync, nc.scalar])
        sweep([6, 7], {0: [6], 1: [7]}, [nc.sync, nc.scalar])

    nc.compile()
    return nc


def _inputs_for_cores(z: np.ndarray, window: np.ndarray):
    amat = _CACHE.get("amat")
    if amat is None:
        amat = _amat()
        _CACHE["amat"] = amat
    w64 = window.astype(np.float64)
    # window and the interior 1/ws (= 0.5) folded into A host-side
    awn = np.ascontiguousarray((amat * (w64 * 0.5)[None, :]).astype(BF16))
    # ch1 taps: rows 0-3 = w-quarters * 0.5/N, rows 4-7 = same * (-1)^r
    w4 = w64.reshape(4, 256) * (0.5 / N_FFT)
    alt = 1.0 - 2.0 * (np.arange(256) % 2)
    taps = np.ascontiguousarray(
        np.concatenate([w4, w4 * alt[None, :]], 0).astype(BF16))

    in_maps = []
    for c in range(N_CORES):
        G = 1024 * c - 1  # global frame index of slot 0
        X = np.zeros((1034, F_SLOTS), np.float32)
        lo, hi = max(0, G), min(T_FRAMES, G + F_SLOTS)
        s0, s1 = lo - G, hi - G
        X[0:513, s0:s1] = z[0, :, lo:hi]
        X[513:1024, s0:s1] = z[1, 1:512, lo:hi]
        X[1024, s0:s1] = z[1, 0, lo:hi]
        X[1025, s0:s1] = z[1, 512, lo:hi]
        for q in range(4):  # pre-shifted zi0/zi512 rows for the ch1 lhsT
            X[1026 + q, 0:NB] = X[1024, 3 - q:3 - q + NB]
            X[1030 + q, 0:NB] = X[1025, 3 - q:3 - q + NB]
        in_maps.append({
            "x": X.astype(BF16),
            "awn": awn,
            "taps": taps,
        })
    return in_maps


def kernel(z: np.ndarray, window: np.ndarray) -> np.ndarray:
    from concourse.bass_utils import run_bass_kernel_spmd

    z = np.asarray(z, dtype=np.float32)
    window = np.asarray(window, dtype=np.float32)

    nc = _CACHE.get("nc")
    if nc is None:
        nc = _build_nc()
        _CACHE["nc"] = nc

    in_maps = _inputs_for_cores(z, window)
    res = run_bass_kernel_spmd(nc, in_maps, list(range(N_CORES)))

    parts = []
    for c in range(N_CORES):
        nb = NB if c < N_CORES - 1 else NB - 1
        o = res.results[c]["out"]  # [2, NB, 256] bf16
        parts.append(o[:, :nb, :].reshape(2, -1).astype(np.float32))
    out = np.concatenate(parts, axis=1)
    # edge fixup: first/last 256 samples see a 3-frame window sum
    # (2 - w[768+r] and 2 - w[r]); the kernel normalized by 2 everywhere.
    w64 = window.astype(np.float64)
    out[:, :256] *= (2.0 / (2.0 - w64[768:1024])).astype(np.float32)
    out[:, -256:] *= (2.0 / (2.0 - w64[0:256])).astype(np.float32)
    return np.ascontiguousarray(out)
